# revision 9
# baseline (speedup 1.0000x reference)
"""GQA kernel for trn2, 8 cores: DP over batch (2) x TP over kv-head groups (4).

Each core computes, for its (batch b, kv-group g):
  - qkv projection for its 4 q-heads + 1 kv-head (q pre-scaled by 1/sqrt(dk))
  - RoPE on q/k
  - full (non-causal) attention for the 4 q-heads vs its kv-head
  - partial out-projection with its 2048 rows of W_out
Host sums the 4 per-group partials per batch and adds bias.

Matmul operands are bf16 (PE runs fp32 at 1/4 rate; bf16 is full rate).
Accumulation and softmax statistics stay fp32.

x is transposed on the host (free) so the PE does no transposes.
Softmax denominator: DVE tree-reduction over key chunks + one [1,512]
ones-matmul per pair (instead of a 16-deep ones-matmul chain).

Self-contained: hardcodes all shapes. kernel(**inputs) -> np.ndarray.
"""

import math
from contextlib import ExitStack

import numpy as np
import ml_dtypes

import concourse.bass as bass
import concourse.bacc as bacc
import concourse.tile as tile
import concourse.mybir as mybir
from concourse.bass_utils import run_bass_kernel_spmd

F32 = mybir.dt.float32
BF16 = mybir.dt.bfloat16
L = 2048          # sequence length
D = 2048          # d_model
DK = 128          # head dim (q/k)
DV = 512          # head dim (v)
NHQ = 4           # q heads per core
CQK = NHQ * DK + DK   # 640 qk projection cols per core
NI = 4            # query chunks of 512
NJ = 16           # key chunks of 128
NDCH = 16         # d_model chunks of 128

_NC_CACHE = {}


def build_nc():
    if "nc" in _NC_CACHE:
        return _NC_CACHE["nc"]
    nc = bacc.Bacc("TRN2", target_bir_lowering=False, debug=False)

    xt_d = nc.dram_tensor("xt", [D, L], BF16, kind="ExternalInput")
    wqk_d = nc.dram_tensor("wqk", [D, CQK], BF16, kind="ExternalInput")
    wv_d = nc.dram_tensor("wv", [D, DV], BF16, kind="ExternalInput")
    wo_d = nc.dram_tensor("wo", [NHQ * DV, D], BF16, kind="ExternalInput")
    cos_d = nc.dram_tensor("cost", [DK, L], F32, kind="ExternalInput")
    sin_d = nc.dram_tensor("sint", [DK, L], F32, kind="ExternalInput")
    out_d = nc.dram_tensor("out", [L, D], F32, kind="ExternalOutput")

    EXP = mybir.ActivationFunctionType.Exp

    xt_ap = xt_d.ap().rearrange("(t p) l -> p t l", p=128)

    with ExitStack() as ctx:
        tc = ctx.enter_context(tile.TileContext(nc))
        # pools
        persist = ctx.enter_context(tc.tile_pool(name="persist", bufs=1))
        psS = ctx.enter_context(tc.tile_pool(name="psS", bufs=5, space="PSUM"))
        psA = ctx.enter_context(tc.tile_pool(name="psA", bufs=3, space="PSUM"))

        ones = persist.tile([128, 1], BF16)
        nc.vector.memset(ones, 1.0)

        qT = persist.tile([128, NHQ, L], BF16)      # [dk, h, pos]
        kT = persist.tile([128, L], BF16)           # [dk, pos]
        v_sb = persist.tile([128, NJ, DV], BF16)    # [key_in_chunk, key_chunk, e]

        # ---------------- Phase B: qkv projection + rope --------------------
        with tc.tile_pool(name="pb1", bufs=1) as pb1, \
             tc.tile_pool(name="pb2", bufs=2) as pb2:
            wv_sb = pb1.tile([128, NDCH, DV], BF16)
            # qk projection weights: loaded ONCE (shared across all i), with
            # 1280B-contiguous descriptors (full 640-col rows)
            wqk_sb = pb1.tile([128, NDCH, CQK], BF16)
            wqk_ap = wqk_d.ap().rearrange("(t p) c -> p t c", p=128)

            def load_xcs(i, nsub=4):
                """Prefetch x^T + rope-table chunks for query chunk i."""
                isl = slice(i * 512, (i + 1) * 512)
                w = NDCH // nsub
                xT = pb2.tile([128, NDCH, 512], BF16, tag="xT")
                for tg in range(nsub):
                    nc.gpsimd.dma_start(
                        out=xT[:, w * tg:w * tg + w, :],
                        in_=xt_ap[:, w * tg:w * tg + w, isl])
                # rope tables on the (idle) scalar queue so the first rope
                # isn't serialized behind the x/w loads
                csc = pb2.tile([128, 512], F32, tag="cos")
                nc.scalar.dma_start(out=csc, in_=cos_d.ap()[:, isl])
                ssc = pb2.tile([128, 512], F32, tag="sin")
                nc.scalar.dma_start(out=ssc, in_=sin_d.ap()[:, isl])
                return xT, csc, ssc

            cur = load_xcs(0, nsub=8)
            # v weights + qk weights land behind the first x/rope chunk
            for tg in range(4):
                nc.gpsimd.dma_start(
                    out=wv_sb[:, 4 * tg:4 * tg + 4, :],
                    in_=wv_d.ap().rearrange("(t p) c -> p t c", p=128)
                        [:, 4 * tg:4 * tg + 4, :])
            for tg in range(8):
                nc.sync.dma_start(out=wqk_sb[:, 2 * tg:2 * tg + 2, :],
                                  in_=wqk_ap[:, 2 * tg:2 * tg + 2, :])

            for i in range(NI):
                isl = slice(i * 512, (i + 1) * 512)
                xT, csc, ssc = cur
                if i < NI - 1:
                    cur = load_xcs(i + 1)

                # q/k projection + rope (c = 0..3 q heads, c = 4 is k)
                for c in range(5):
                    ps = psA.tile([128, 512], F32, tag="acc")
                    for t in range(NDCH):
                        nc.tensor.matmul(
                            ps, lhsT=wqk_sb[:, t, c * 128:(c + 1) * 128],
                            rhs=xT[:, t, :],
                            start=(t == 0), stop=(t == NDCH - 1))
                    dest = qT[:, c, isl] if c < NHQ else kT[:, isl]
                    tmp = pb2.tile([128, 512], F32, tag="rope")
                    nc.vector.tensor_mul(tmp[0:64, :], ps[64:128, :], ssc[0:64, :])
                    nc.vector.tensor_mul(tmp[64:128, :], ps[0:64, :], ssc[64:128, :])
                    tmp2 = pb2.tile([128, 512], F32, tag="rope2")
                    nc.vector.tensor_mul(tmp2, ps, csc)
                    nc.vector.tensor_sub(dest[0:64, :], tmp2[0:64, :], tmp[0:64, :])
                    nc.vector.tensor_add(dest[64:128, :], tmp2[64:128, :],
                                         tmp[64:128, :])

                # v projection for these 4 key chunks
                for lsub in range(4):
                    ps = psA.tile([128, 512], F32, tag="acc")
                    for t in range(NDCH):
                        nc.tensor.matmul(
                            ps, lhsT=xT[:, t, lsub * 128:(lsub + 1) * 128],
                            rhs=wv_sb[:, t, :],
                            start=(t == 0), stop=(t == NDCH - 1))
                    nc.scalar.copy(out=v_sb[:, i * 4 + lsub, :], in_=ps)

        # ---------------- Phase C+D: attention + fused out-projection -------
        # software-pipelined: S/exp of pair k+1 is emitted before ones/PV of
        # pair k so ACT exp latency hides under PE's PV matmuls.
        with tc.tile_pool(name="pc1", bufs=1) as pc1, \
             tc.tile_pool(name="pc2", bufs=2) as pc2:
            ctxTs = {}

            def emit_s_exp(i, h):
                isl = slice(i * 512, (i + 1) * 512)
                expS = pc2.tile([128, NJ, 512], BF16, tag="expS")
                sacc = pc2.tile([128, 512], BF16, tag="sacc")
                for j in range(NJ):
                    ps = psS.tile([128, 512], F32, tag="stream")
                    nc.tensor.matmul(ps, lhsT=kT[:, j * 128:(j + 1) * 128],
                                     rhs=qT[:, h, isl])
                    nc.scalar.activation(out=expS[:, j, :], in_=ps, func=EXP)
                    # running denominator partial sum on DVE (free axis = j)
                    if j == 1:
                        nc.vector.tensor_add(sacc, expS[:, 0, :], expS[:, 1, :])
                    elif j > 1:
                        nc.vector.tensor_add(sacc, sacc, expS[:, j, :])
                return expS, sacc

            def emit_pv(i, h, expS, sacc):
                # cross-partition part of the denominator: one 512-row matmul
                pso = psA.tile([1, 512], F32, tag="acc")
                nc.tensor.matmul(pso, lhsT=ones[:, 0:1], rhs=sacc)
                recip = pc1.tile([1, 512], F32, tag="recip")
                nc.vector.reciprocal(recip, pso)
                rb = pc2.tile([128, 512], F32, tag="rb")
                nc.gpsimd.partition_broadcast(rb, recip)
                ctxT = pc1.tile([128, 4, 512], BF16, tag=f"ctx{h}")
                for ec in range(4):
                    ps = psA.tile([128, 512], F32, tag="acc")
                    for j in range(NJ):
                        nc.tensor.matmul(
                            ps, lhsT=v_sb[:, j, ec * 128:(ec + 1) * 128],
                            rhs=expS[:, j, :],
                            start=(j == 0), stop=(j == NJ - 1))
                    nc.vector.tensor_mul(ctxT[:, ec, :], ps, rb)
                ctxTs[h] = ctxT

            wo_ap = wo_d.ap().rearrange("(t p) c -> p t c", p=128)

            def emit_outproj(i):
                for dm in range(4):
                    wo_t = pc2.tile([128, NDCH, 512], BF16, tag="wo")
                    for half in range(2):
                        nc.sync.dma_start(
                            out=wo_t[:, 8 * half:8 * half + 8, :],
                            in_=wo_ap[:, 8 * half:8 * half + 8,
                                      dm * 512:(dm + 1) * 512])
                    for lsub in range(4):
                        ps = psA.tile([128, 512], F32, tag="acc")
                        for h in range(NHQ):
                            for ec in range(4):
                                t = h * 4 + ec
                                nc.tensor.matmul(
                                    ps,
                                    lhsT=ctxTs[h][:, ec,
                                                  lsub * 128:(lsub + 1) * 128],
                                    rhs=wo_t[:, t, :],
                                    start=(t == 0), stop=(t == 15))
                        ost = pc2.tile([128, 512], F32, tag="ost")
                        nc.scalar.copy(out=ost, in_=ps)
                        l0 = i * 512 + lsub * 128
                        nc.scalar.dma_start(
                            out=out_d.ap()[l0:l0 + 128,
                                           dm * 512:(dm + 1) * 512],
                            in_=ost)

            pairs = [(i, h) for i in range(NI) for h in range(NHQ)]
            prev = None
            for (i, h) in pairs:
                cur = (i, h, *emit_s_exp(i, h))
                if prev is not None:
                    pi, ph, pexp, psacc = prev
                    emit_pv(pi, ph, pexp, psacc)
                    if ph == NHQ - 1:
                        emit_outproj(pi)
                prev = cur
            pi, ph, pexp, psacc = prev
            emit_pv(pi, ph, pexp, psacc)
            emit_outproj(pi)

    nc.compile()
    _NC_CACHE["nc"] = nc
    return nc


def make_core_inputs(x, W_attn, W_out):
    """Split full inputs into 8 per-core input maps (core = b*4 + g)."""
    Q_DIM = 2048
    K_DIM = 512
    scale = np.float32(1.0 / math.sqrt(DK))
    bf = ml_dtypes.bfloat16

    # rope tables, mirroring the fp32 reference computation
    inv_freq = (np.float32(1.0) /
                (np.float32(10000.0) **
                 (np.arange(0, DK, 2, dtype=np.float32) / np.float32(DK))))
    freqs = np.arange(L, dtype=np.float32)[:, None] * inv_freq[None, :]  # [L,64]
    ang = np.concatenate([freqs, freqs], axis=-1)  # [L, 128]
    cosT = np.ascontiguousarray(np.cos(ang).T.astype(np.float32))  # [128, L]
    sinT = np.ascontiguousarray(np.sin(ang).T.astype(np.float32))

    xts = [np.ascontiguousarray(x[b].T).astype(bf) for b in range(2)]

    in_maps = []
    for core in range(8):
        b, g = divmod(core, 4)
        wq = (W_attn[:, 512 * g:512 * (g + 1)] * scale)
        wk = W_attn[:, Q_DIM + 128 * g:Q_DIM + 128 * (g + 1)]
        wqk = np.ascontiguousarray(
            np.concatenate([wq, wk], axis=1)).astype(bf)
        wv = np.ascontiguousarray(W_attn[:, Q_DIM + K_DIM + 512 * g:
                                         Q_DIM + K_DIM + 512 * (g + 1)]).astype(bf)
        wo = np.ascontiguousarray(W_out[2048 * g:2048 * (g + 1), :]).astype(bf)
        in_maps.append({
            "xt": xts[b],
            "wqk": wqk,
            "wv": wv,
            "wo": wo,
            "cost": cosT,
            "sint": sinT,
        })
    return in_maps


def kernel(x, W_attn, W_out, b_out, _trace=False, _trace_cores=None):
    x = np.asarray(x)
    W_attn = np.asarray(W_attn)
    W_out = np.asarray(W_out)
    b_out = np.asarray(b_out)
    nc = build_nc()
    in_maps = make_core_inputs(x, W_attn, W_out)
    res = run_bass_kernel_spmd(
        nc, in_maps, core_ids=list(range(8)),
        trace=_trace, trace_cores=_trace_cores)
    parts = [res.results[c]["out"] for c in range(8)]
    out = np.empty((2, L, D), dtype=np.float32)
    for b in range(2):
        acc = parts[4 * b].astype(np.float32)
        for g in range(1, 4):
            acc = acc + parts[4 * b + g]
        out[b] = acc + b_out[None, :].astype(np.float32)
    if _trace:
        kernel._last_results = res
    return out
